# revision 1
# baseline (speedup 1.0000x reference)
"""CRF-RNN (crfasrnn) Bass kernel for 8 Trainium2 NeuronCores.

N=8192 voxels, L=21 labels. Each core owns a 1024-voxel column block of the
two NxN Gaussian kernels K_sp/K_bi. K_sp is built once into SBUF (bf16,
128KB/partition); K_bi is recomputed per mean-field iteration (both cached
in bf16 would not fit), fused with the filtering matmul:

  gram matmul -> exp (ACT, j-side -0.5*||f||^2 as exact fp32 per-partition
  bias) -> filtering matmul (S[:,Rc] = [q; ones] @ K[:,Rc]; the ones row
  yields the normalizer n for free).

The gram runs at 1 PE cycle/row (vs 4 for fp32) via an exact two-limb bf16
split: features f = hi + lo, rows [Uh;Uh;Ul;Ul] x [Vh;Vl;Vh;Vl] give all
hi/lo cross products in one bf16 matmul (fp32 PSUM accumulate). The i-side
-0.5*||f||^2 enters as two bf16 limb rows: its truncation error is a pure
per-column factor that cancels exactly in S/n; it only needs to bound the
exponent. q is bf16 (both matmul operands must share a dtype; bf16 x fp8
crashes the exec unit, fp32 x fp8 is rejected at build).

The 21x21 label-compatibility matmuls are folded host-side (A = compat@W_sp,
B = compat@W_bi) and fused with the S-transpose into one PE op per tile.
Per iteration the cores all-gather their (1024,21) bf16 q slices; iteration
0 softmaxes the full logits locally instead. Softmax over 21 runs along the
free dim with activation accum_out (fused exp+sum), skipping max-subtraction
(cur is bounded, exp stays finite in fp32).

Hard-won constraints: two PE matmul accumulation groups into one PSUM bank
crash the NEFF (every matmul gets its own PSUM tensor); a 3D-sliced
activation bias AP also crashes the exec unit (bias tiles must be 2D).
"""

import sys

sys.path.insert(0, "/opt/trn_rl_repo")

import numpy as np

NUM_CORES = 8
GAMMA, ALPHA, BETA = 3.0, 160.0, 3.0
NUM_ITERS = 5
L = 21
D, H, W = 8, 32, 32
N = D * H * W          # 8192
R = N // NUM_CORES     # 1024 columns per core
NB = N // 128          # 64 j tiles
RT = R // 128          # 8 i tiles per core
FSP = 14               # 3 spatial features x4 hi/lo cross limbs + 2 sq rows
FBI = 26               # 6 bilateral features x4 hi/lo cross limbs + 2 sq rows
LQ = 33                # q rows padded so the ones row sits at partition 32

_CACHE = {}


def _build(num_iters=NUM_ITERS, sim=False, qdt="bfloat16", kdt="bfloat16",
           cache_k=False, hybrid=True):
    key = ("nc", num_iters, sim, qdt, kdt, cache_k, hybrid)
    if key in _CACHE:
        return _CACHE[key]
    NUM_ITERS_LOCAL = num_iters

    import concourse.bacc as bacc
    import concourse.mybir as mybir
    import concourse.tile as tile

    f32 = mybir.dt.float32
    QDT = getattr(mybir.dt, qdt)
    KDT = getattr(mybir.dt, kdt)
    EXP = mybir.ActivationFunctionType.Exp
    CPY = mybir.ActivationFunctionType.Copy
    MUL = mybir.AluOpType.mult
    ADD = mybir.AluOpType.add

    nc = bacc.Bacc(
        "TRN2", target_bir_lowering=False, debug=False,
        num_devices=1 if sim else NUM_CORES,
    )

    bf16 = mybir.dt.bfloat16
    usp = nc.dram_tensor("usp", [FSP, N], bf16, kind="ExternalInput").ap()
    vsp = nc.dram_tensor("vsp", [FSP, R], bf16, kind="ExternalInput").ap()
    ubi = nc.dram_tensor("ubi", [FBI, N], bf16, kind="ExternalInput").ap()
    vbi = nc.dram_tensor("vbi", [FBI, R], bf16, kind="ExternalInput").ap()
    sqj = nc.dram_tensor("sqj", [128, 2 * NB], f32, kind="ExternalInput").ap()
    unt = nc.dram_tensor("unt", [R, L], f32, kind="ExternalInput").ap()
    lgt = nc.dram_tensor("lgt", [N, L], f32, kind="ExternalInput").ap()
    wat = nc.dram_tensor("wat", [L, L], f32, kind="ExternalInput").ap()
    wbt = nc.dram_tensor("wbt", [L, L], f32, kind="ExternalInput").ap()
    outq = nc.dram_tensor("outq", [R, L], f32, kind="ExternalOutput").ap()

    qsl = nc.dram_tensor("qsl", [R, L], QDT).ap()
    qfull = nc.dram_tensor("qfull", [N, L], QDT, addr_space="Shared").ap()

    with tile.TileContext(nc) as tc:
        with (
            tc.tile_pool(name="const", bufs=1) as cpool,
            tc.tile_pool(name="ssb", bufs=2) as wpool,
            tc.tile_pool(name="ustream", bufs=4) as upool,
            tc.tile_pool(name="small", bufs=4) as spool,
            tc.tile_pool(name="gps", bufs=2, space="PSUM") as gpool,
            tc.tile_pool(name="sps", bufs=2, space="PSUM") as s_pool,
        ):
            vsp_sb = cpool.tile([FSP, R], bf16)
            nc.sync.dma_start(vsp_sb[:], vsp)
            vbi_sb = cpool.tile([FBI, R], bf16)
            nc.sync.dma_start(vbi_sb[:], vbi)
            sqj_sb = cpool.tile([128, 2 * NB], f32)
            nc.sync.dma_start(sqj_sb[:], sqj)
            wat_sb = cpool.tile([L, L], f32)
            nc.sync.dma_start(wat_sb[:], wat)
            wbt_sb = cpool.tile([L, L], f32)
            nc.sync.dma_start(wbt_sb[:], wbt)
            unt_sb = cpool.tile([128, RT, L], f32)
            nc.sync.dma_start(unt_sb[:], unt.rearrange("(t p) l -> p t l", p=128))
            one1 = cpool.tile([LQ, 1], f32)
            nc.vector.memset(one1[:], 1.0)

            qT = cpool.tile([128, NB, LQ], QDT)
            nc.vector.memset(qT[:, :, LQ - 1], 1.0)
            curT = cpool.tile([128, RT, L], f32)

            build_list = []
            if cache_k or hybrid:
                # persistent K_sp column-block, built once (streamed lhsT)
                ksp_st = cpool.tile([128, NB, R], KDT)
                build_list.append((usp, vsp_sb, ksp_st, FSP, 0))
            if cache_k:
                kbi_st = cpool.tile([128, NB, R], KDT)
                build_list.append((ubi, vbi_sb, kbi_st, FBI, 1))
            for jt in range(NB):
                for u_dram, v_sb, store, fdim, ki in build_list:
                    u_t = upool.tile([fdim, 128], bf16, tag="u")
                    nc.sync.dma_start(
                        u_t[:], u_dram[:, jt * 128 : (jt + 1) * 128]
                    )
                    g = gpool.tile([128, 1024], f32, tag="g")
                    for h in range(2):
                        nc.tensor.matmul(
                            g[:, h * 512 : (h + 1) * 512], u_t[:],
                            v_sb[:, h * 512 : (h + 1) * 512],
                            start=True, stop=True,
                        )
                    nc.scalar.activation(
                        store[:, jt, :], g[:], EXP,
                        bias=sqj_sb[:, ki * NB + jt : ki * NB + jt + 1],
                    )
            if not (cache_k or hybrid):
                usp_sb = cpool.tile([FSP, N], bf16)
                nc.sync.dma_start(usp_sb[:], usp)
            if not cache_k:
                ubi_sb = cpool.tile([FBI, N], bf16)
                nc.sync.dma_start(ubi_sb[:], ubi)

            # iteration-0 softmax of the full logits: local, no gather needed
            lg_sb = cpool.tile([128, NB, L], f32)
            nc.sync.dma_start(lg_sb[:], lgt.rearrange("(t p) l -> p t l", p=128))
            for jt in range(NB):
                ssum = spool.tile([128, 1], f32, tag="sum")
                rsum = spool.tile([128, 1], f32, tag="rec")
                nc.scalar.activation(
                    qT[:, jt, 0:L], lg_sb[:, jt], EXP, accum_out=ssum[:]
                )
                nc.vector.reciprocal(rsum[:], ssum[:])
                nc.vector.tensor_scalar_mul(qT[:, jt, 0:L], qT[:, jt, 0:L], rsum[:])

            for step in range(NUM_ITERS_LOCAL):
                ssp_ps = s_pool.tile([LQ, R], f32, tag="s")
                sbi_ps = s_pool.tile([LQ, R], f32, tag="s")
                if hybrid and not cache_k:
                    # software pipeline: the bi gram/exp chain is
                    # q-independent, so keep PF tiles of lookahead emitted
                    # ahead of the q-consuming matmuls -- PE/ACT work
                    # through the all-gather latency instead of stalling
                    def emit_k(jt):
                        g = gpool.tile([128, 1024], f32, tag="g", name="g")
                        for h in range(2):
                            nc.tensor.matmul(
                                g[:, h * 512 : (h + 1) * 512],
                                ubi_sb[:, jt * 128 : (jt + 1) * 128],
                                vbi_sb[:, h * 512 : (h + 1) * 512],
                                start=True, stop=True,
                            )
                        kt = upool.tile([128, 1024], KDT, tag="kt", name="kt",
                                        bufs=10)
                        nc.scalar.activation(
                            kt[:], g[:], EXP,
                            bias=sqj_sb[:, NB + jt : NB + jt + 1],
                        )
                        return kt

                    PF = 8
                    ktq = [emit_k(i) for i in range(PF)]
                    for jt in range(NB):
                        if jt + PF < NB:
                            ktq.append(emit_k(jt + PF))
                        kt = ktq.pop(0)
                        for h in range(2):
                            nc.tensor.matmul(
                                ssp_ps[:, h * 512 : (h + 1) * 512],
                                qT[:, jt, :],
                                ksp_st[:, jt, h * 512 : (h + 1) * 512],
                                start=(jt == 0), stop=(jt == NB - 1),
                            )
                            nc.tensor.matmul(
                                sbi_ps[:, h * 512 : (h + 1) * 512],
                                qT[:, jt, :],
                                kt[:, h * 512 : (h + 1) * 512],
                                start=(jt == 0), stop=(jt == NB - 1),
                            )
                for jt in range(NB if not (hybrid and not cache_k) else 0):
                    cached = [(ksp_st, ssp_ps)] if (cache_k or hybrid) else []
                    if cache_k:
                        cached.append((kbi_st, sbi_ps))
                    for store, s_ps in cached:
                        for h in range(2):
                            nc.tensor.matmul(
                                s_ps[:, h * 512 : (h + 1) * 512],
                                qT[:, jt, :],
                                store[:, jt, h * 512 : (h + 1) * 512],
                                start=(jt == 0),
                                stop=(jt == NB - 1),
                            )
                    recomp = []
                    if not (cache_k or hybrid):
                        recomp.append((usp_sb, vsp_sb, ssp_ps, 0))
                    if not cache_k:
                        recomp.append((ubi_sb, vbi_sb, sbi_ps, 1))
                    for u_sb, v_sb, s_ps, ki in recomp:
                            for h in range(2):
                                g = gpool.tile([128, 512], f32, tag="g")
                                nc.tensor.matmul(
                                    g[:],
                                    u_sb[:, jt * 128 : (jt + 1) * 128],
                                    v_sb[:, h * 512 : (h + 1) * 512],
                                    start=True, stop=True,
                                )
                                kt = upool.tile([128, 512], KDT, tag="kt")
                                nc.scalar.activation(
                                    kt[:], g[:], EXP,
                                    bias=sqj_sb[:, ki * NB + jt : ki * NB + jt + 1],
                                )
                                nc.tensor.matmul(
                                    s_ps[:, h * 512 : (h + 1) * 512],
                                    qT[:, jt, :],
                                    kt[:],
                                    start=(jt == 0),
                                    stop=(jt == NB - 1),
                                )

                ssp_sb = wpool.tile([LQ, R], f32, tag="ssb")
                sbi_sb = wpool.tile([LQ, R], f32, tag="ssb")
                nc.vector.tensor_copy(ssp_sb[:], ssp_ps[:])
                nc.vector.tensor_copy(sbi_sb[:], sbi_ps[:])
                last = step == NUM_ITERS_LOCAL - 1
                for it in range(RT):
                    msp = s_pool.tile([128, L], f32, tag="s", name="msp")
                    mbi = s_pool.tile([128, L], f32, tag="s", name="mbi")
                    nsp = gpool.tile([128, 1], f32, tag="g")
                    nbi = gpool.tile([128, 1], f32, tag="g")
                    lo, hi = it * 128, (it + 1) * 128
                    # fused transpose + label matmul; n^T via ones column
                    nc.tensor.matmul(
                        msp[:], ssp_sb[0:L, lo:hi], wat_sb[:],
                        start=True, stop=True,
                    )
                    nc.tensor.matmul(
                        nsp[:], ssp_sb[LQ - 1 : LQ, lo:hi],
                        one1[LQ - 1 : LQ, :], start=True, stop=True,
                    )
                    nc.tensor.matmul(
                        mbi[:], sbi_sb[0:L, lo:hi], wbt_sb[:],
                        start=True, stop=True,
                    )
                    nc.tensor.matmul(
                        nbi[:], sbi_sb[LQ - 1 : LQ, lo:hi],
                        one1[LQ - 1 : LQ, :], start=True, stop=True,
                    )
                    rsp = spool.tile([128, 1], f32, tag="rn")
                    rbi = spool.tile([128, 1], f32, tag="rn")
                    nc.vector.reciprocal(rsp[:], nsp[:])
                    nc.vector.reciprocal(rbi[:], nbi[:])
                    tmp = spool.tile([128, L], f32, tag="tmp")
                    nc.vector.scalar_tensor_tensor(
                        tmp[:], msp[:], rsp[:], unt_sb[:, it], op0=MUL, op1=ADD
                    )
                    nc.vector.scalar_tensor_tensor(
                        curT[:, it], mbi[:], rbi[:], tmp[:], op0=MUL, op1=ADD
                    )
                    ex = spool.tile([128, L], f32, tag="ex")
                    ssum = spool.tile([128, 1], f32, tag="sum")
                    rsum = spool.tile([128, 1], f32, tag="rec")
                    nc.scalar.activation(ex[:], curT[:, it], EXP, accum_out=ssum[:])
                    nc.vector.reciprocal(rsum[:], ssum[:])
                    if last:
                        nc.vector.tensor_scalar_mul(ex[:], ex[:], rsum[:])
                        nc.sync.dma_start(outq[lo:hi, :], ex[:])
                    else:
                        exq = spool.tile([128, L], QDT, tag="exq")
                        nc.vector.tensor_scalar_mul(exq[:], ex[:], rsum[:])
                        nc.sync.dma_start(qsl[lo:hi, :], exq[:])
                if not last:
                    if sim:
                        # timing stand-in for the all-gather: move ~the same
                        # bytes through DRAM locally
                        for c in range(NUM_CORES):
                            nc.sync.dma_start(qfull[c * R : (c + 1) * R, :], qsl)
                    else:
                        nc.gpsimd.collective_compute(
                            "AllGather",
                            mybir.AluOpType.bypass,
                            replica_groups=[list(range(NUM_CORES))],
                            ins=[qsl.opt()],
                            outs=[qfull.opt()],
                        )
                    # chunked reload: mains on early j-tiles start while
                    # later chunks are still loading
                    qf3 = qfull.rearrange("(t p) l -> p t l", p=128)
                    for c4 in range(4):
                        nc.sync.dma_start(
                            qT[:, c4 * 16 : (c4 + 1) * 16, 0:L],
                            qf3[:, c4 * 16 : (c4 + 1) * 16, :],
                        )

    nc.compile()
    _CACHE[key] = nc
    return nc


def _host_inputs(image, logits, unary, spatial_ker_weights, bilateral_ker_weights,
                 compatibility_matrix):
    img = np.asarray(image, np.float32)[0].reshape(3, N)
    zz, yy, xx = np.meshgrid(
        np.arange(D), np.arange(H), np.arange(W), indexing="ij"
    )
    pos = np.stack([zz, yy, xx]).reshape(3, N).astype(np.float32)

    import ml_dtypes

    def to_bf16(x):
        return x.astype(ml_dtypes.bfloat16).astype(np.float32)

    def uv(feats):
        # two-limb bf16 split: f = hi + lo (+ dropped 2^-16 residual)
        fh = to_bf16(feats)
        fl = to_bf16(feats - fh)
        ft = fh + fl                     # the features the device actually uses
        sq = (ft * ft).sum(0, dtype=np.float64).astype(np.float32)
        sh = to_bf16(-0.5 * sq)          # i-side sq limbs; truncation cancels in S/n
        sl = to_bf16(-0.5 * sq - sh)
        ones = np.ones((1, N), np.float32)
        u = np.concatenate([fh, fh, fl, fl, ones, ones], 0)
        v = np.concatenate([fh, fl, fh, fl, sh[None], sl[None]], 0)
        bf = ml_dtypes.bfloat16
        return (np.ascontiguousarray(u).astype(bf),
                np.ascontiguousarray(v).astype(bf), sq)

    u_sp, v_sp, sq_sp_ = uv(pos / GAMMA)
    u_bi, v_bi, sq_bi_ = uv(np.concatenate([pos / ALPHA, img / BETA], 0))
    # exact fp32 j-side bias, laid out (p, kernel, jtile)
    sqj_np = np.stack([-0.5 * sq_sp_, -0.5 * sq_bi_], 0)       # (2, N)
    sqj_np = sqj_np.reshape(2, NB, 128).transpose(2, 0, 1)     # (128, 2, NB)
    sqj_np = np.ascontiguousarray(sqj_np.reshape(128, 2 * NB))

    cm = np.asarray(compatibility_matrix, np.float32)
    wa_t = np.ascontiguousarray((cm @ np.asarray(spatial_ker_weights, np.float32)).T)
    wb_t = np.ascontiguousarray((cm @ np.asarray(bilateral_ker_weights, np.float32)).T)
    un_t = np.ascontiguousarray(np.asarray(unary, np.float32)[0].reshape(L, N).T)
    lg_t = np.ascontiguousarray(np.asarray(logits, np.float32)[0].reshape(L, N).T)

    maps = []
    for c in range(NUM_CORES):
        cols = slice(c * R, (c + 1) * R)
        maps.append({
            "usp": u_sp,
            "vsp": np.ascontiguousarray(v_sp[:, cols]),
            "ubi": u_bi,
            "vbi": np.ascontiguousarray(v_bi[:, cols]),
            "unt": np.ascontiguousarray(un_t[cols]),
            "lgt": lg_t,
            "wat": wa_t,
            "wbt": wb_t,
            "sqj": sqj_np,
        })
    return maps


def kernel(**inputs):
    from concourse.bass_utils import run_bass_kernel_spmd

    nc = _build()
    in_maps = _host_inputs(**inputs)
    res = run_bass_kernel_spmd(nc, in_maps, core_ids=list(range(NUM_CORES)))
    full = np.concatenate([res.results[c]["outq"] for c in range(NUM_CORES)], 0)
    return np.ascontiguousarray(full.T).reshape(1, L, D, H, W).astype(np.float32)



# revision 3
# speedup vs baseline: 1.5672x; 1.5672x over previous
"""CRF-RNN (crfasrnn) Bass kernel for 8 Trainium2 NeuronCores — v2.

N=8192 voxels, L=21 labels, 5 mean-field iterations. Each core owns one
z-plane (R=1024 voxels) of the output columns.

Two structural wins over v1 (dense K_sp + per-iteration K_bi recompute):

1. K_sp is separable: exp(-||dp||^2/(2g^2)) = Gz (x) Gy (x) Gx (Kronecker).
   The spatial filter collapses to a DVE z-mix (8 weighted adds of q
   tiles, weights = the core's Gz row) followed by an 8-k-tile plane
   matmul against Kp = Gy (x) Gx (1024x1024, bf16 in SBUF). 8192 PE
   rows/iter instead of 65536. The spatial normalizer 1/n_sp is a pure
   geometry constant, precomputed host-side.

2. With no dense K_sp to store, K_bi (bf16, 128KB/partition) fits in
   SBUF, built once by the exact two-limb bf16 gram (see v1 notes) and
   reused all 5 iterations: no per-iteration gram matmul or exp.

The K_bi build (PE gram + ACT exp, ~427/857ns per j-tile) is software-
pipelined with iteration 0's softmax and bi-filter matmuls so PE and ACT
run concurrently instead of serially. The bi normalizer n_bi comes free
as a ones row in the q lhsT (PE cost is free-dim-driven, extra M row is
free). Label-compatibility matmuls are folded host-side (A = compat@W)
and fused with the S-transpose, as in v1.

Hard-won v1 constraints that still apply: every PE accumulation group
needs its own PSUM tensor; ACT bias APs must be 2D; both matmul operands
must share a dtype.
"""

import sys

sys.path.insert(0, "/opt/trn_rl_repo")

import numpy as np

NUM_CORES = 8
GAMMA, ALPHA, BETA = 3.0, 160.0, 3.0
NUM_ITERS = 5
L = 21
D, H, W = 8, 32, 32
N = D * H * W          # 8192
R = N // NUM_CORES     # 1024 columns per core (one z-plane)
NB = N // 128          # 64 j tiles
RT = R // 128          # 8 i tiles per core
PT = R // 128          # 8 plane k-tiles
FBI = 26               # 6 bilateral features x4 hi/lo cross limbs + 2 sq rows
LQ = 33                # q rows padded so the ones row (n_bi) sits at partition 32

_CACHE = {}


def _build(num_iters=NUM_ITERS, sim=False):
    key = ("nc", num_iters, sim)
    if key in _CACHE:
        return _CACHE[key]

    import concourse.bacc as bacc
    import concourse.mybir as mybir
    import concourse.tile as tile

    f32 = mybir.dt.float32
    bf16 = mybir.dt.bfloat16
    EXP = mybir.ActivationFunctionType.Exp
    CPY = mybir.ActivationFunctionType.Copy
    MUL = mybir.AluOpType.mult
    ADD = mybir.AluOpType.add

    nc = bacc.Bacc(
        "TRN2", target_bir_lowering=False, debug=False,
        num_devices=1 if sim else NUM_CORES,
    )

    ubi = nc.dram_tensor("ubi", [FBI, N], bf16, kind="ExternalInput").ap()
    vbi = nc.dram_tensor("vbi", [FBI, R], bf16, kind="ExternalInput").ap()
    sqj = nc.dram_tensor("sqj", [128, NB], f32, kind="ExternalInput").ap()
    kpl = nc.dram_tensor("kpl", [128, PT * R], bf16, kind="ExternalInput").ap()
    gzc = nc.dram_tensor("gzc", [128, D], f32, kind="ExternalInput").ap()
    rsp = nc.dram_tensor("rsp", [128, RT], f32, kind="ExternalInput").ap()
    unt = nc.dram_tensor("unt", [R, L], f32, kind="ExternalInput").ap()
    lgt = nc.dram_tensor("lgt", [N, L], f32, kind="ExternalInput").ap()
    wat = nc.dram_tensor("wat", [L, L], f32, kind="ExternalInput").ap()
    wbt = nc.dram_tensor("wbt", [L, L], f32, kind="ExternalInput").ap()
    outq = nc.dram_tensor("outq", [R, L], f32, kind="ExternalOutput").ap()

    qsl = nc.dram_tensor("qsl", [R, L], bf16).ap()
    qfull = nc.dram_tensor("qfull", [N, L], bf16, addr_space="Shared").ap()

    with tile.TileContext(nc) as tc:
        with (
            tc.tile_pool(name="const", bufs=1) as cpool,
            tc.tile_pool(name="ssb", bufs=2) as wpool,
            tc.tile_pool(name="ustream", bufs=8) as upool,
            tc.tile_pool(name="small", bufs=4) as spool,
            tc.tile_pool(name="gps", bufs=2, space="PSUM") as gpool,
            tc.tile_pool(name="sps", bufs=2, space="PSUM") as s_pool,
        ):
            vbi_sb = cpool.tile([FBI, R], bf16)
            nc.sync.dma_start(vbi_sb[:], vbi)
            sqj_sb = cpool.tile([128, NB], f32)
            nc.sync.dma_start(sqj_sb[:], sqj)
            kpl_sb = cpool.tile([128, PT, R], bf16)
            nc.sync.dma_start(kpl_sb[:], kpl.rearrange("p (t i) -> p t i", t=PT))
            gzc_sb = cpool.tile([128, D], f32)
            nc.sync.dma_start(gzc_sb[:], gzc)
            rsp_sb = cpool.tile([128, RT], f32)
            nc.sync.dma_start(rsp_sb[:], rsp)
            wat_sb = cpool.tile([L, L], f32)
            nc.sync.dma_start(wat_sb[:], wat)
            wbt_sb = cpool.tile([L, L], f32)
            nc.sync.dma_start(wbt_sb[:], wbt)
            unt_sb = cpool.tile([128, RT, L], f32)
            nc.sync.dma_start(unt_sb[:], unt.rearrange("(t p) l -> p t l", p=128))
            lg_sb = cpool.tile([128, NB, L], f32)
            nc.sync.dma_start(lg_sb[:], lgt.rearrange("(t p) l -> p t l", p=128))
            one1 = cpool.tile([LQ, 1], f32)
            nc.vector.memset(one1[:], 1.0)

            qT = cpool.tile([128, NB, LQ], bf16)
            nc.vector.memset(qT[:, :, LQ - 1], 1.0)
            kbi_st = cpool.tile([128, NB, R], bf16)

            def emit_softmax0(jt):
                ssum = spool.tile([128, 1], f32, tag="sum")
                rsum = spool.tile([128, 1], f32, tag="rec")
                nc.scalar.activation(
                    qT[:, jt, 0:L], lg_sb[:, jt], EXP, accum_out=ssum[:]
                )
                nc.vector.reciprocal(rsum[:], ssum[:])
                nc.vector.tensor_scalar_mul(qT[:, jt, 0:L], qT[:, jt, 0:L], rsum[:])

            def emit_build(jt):
                u_t = upool.tile([FBI, 128], bf16, tag="u")
                nc.sync.dma_start(u_t[:], ubi[:, jt * 128 : (jt + 1) * 128])
                g = gpool.tile([128, R], f32, tag="g")
                for h in range(2):
                    nc.tensor.matmul(
                        g[:, h * 512 : (h + 1) * 512], u_t[:],
                        vbi_sb[:, h * 512 : (h + 1) * 512],
                        start=True, stop=True,
                    )
                nc.scalar.activation(
                    kbi_st[:, jt, :], g[:], EXP,
                    bias=sqj_sb[:, jt : jt + 1],
                )

            def emit_bi(sbi_ps, jt):
                for h in range(2):
                    nc.tensor.matmul(
                        sbi_ps[:, h * 512 : (h + 1) * 512],
                        qT[:, jt, :],
                        kbi_st[:, jt, h * 512 : (h + 1) * 512],
                        start=(jt == 0), stop=(jt == NB - 1),
                    )

            for step in range(num_iters):
                sbi_ps = s_pool.tile([LQ, R], f32, tag="s")
                if step == 0:
                    # software pipeline: K_bi build (PE gram + ACT exp) and
                    # iteration-0 softmax run PF j-tiles ahead of the
                    # consuming bi-filter matmuls
                    PF = 6
                    for jt in range(PF):
                        emit_softmax0(jt)
                        emit_build(jt)
                    for jt in range(NB):
                        if jt + PF < NB:
                            emit_softmax0(jt + PF)
                            emit_build(jt + PF)
                        emit_bi(sbi_ps, jt)
                else:
                    for jt in range(NB):
                        emit_bi(sbi_ps, jt)

                # spatial path: z-mix on DVE (runs under the bi matmuls),
                # then the separable plane matmul (8 k-tiles)
                acc = spool.tile([128, PT, L], f32, tag="zm")
                nc.vector.tensor_scalar_mul(
                    acc[:], qT[:, 0:PT, 0:L], gzc_sb[:, 0:1]
                )
                for z in range(1, D - 1):
                    nc.vector.scalar_tensor_tensor(
                        acc[:], qT[:, z * PT : (z + 1) * PT, 0:L],
                        gzc_sb[:, z : z + 1], acc[:], op0=MUL, op1=ADD,
                    )
                bmix = spool.tile([128, PT, L], bf16, tag="bm")
                nc.vector.scalar_tensor_tensor(
                    bmix[:], qT[:, (D - 1) * PT : D * PT, 0:L],
                    gzc_sb[:, D - 1 : D], acc[:], op0=MUL, op1=ADD,
                )
                ssp_ps = s_pool.tile([L, R], f32, tag="s")
                for kt in range(PT):
                    for h in range(2):
                        nc.tensor.matmul(
                            ssp_ps[:, h * 512 : (h + 1) * 512],
                            bmix[:, kt, :],
                            kpl_sb[:, kt, h * 512 : (h + 1) * 512],
                            start=(kt == 0), stop=(kt == PT - 1),
                        )

                ssp_sb = wpool.tile([L, R], f32, tag="ssb")
                sbi_sb = wpool.tile([LQ, R], f32, tag="ssb")
                nc.scalar.activation(ssp_sb[:], ssp_ps[:], CPY)
                nc.vector.tensor_copy(sbi_sb[:], sbi_ps[:])
                last = step == num_iters - 1
                for it in range(RT):
                    msp = s_pool.tile([128, L], f32, tag="s", name="msp")
                    mbi = s_pool.tile([128, L], f32, tag="s", name="mbi")
                    nbi = gpool.tile([128, 1], f32, tag="g")
                    lo, hi = it * 128, (it + 1) * 128
                    # fused transpose + label matmul; n_bi^T via ones column
                    nc.tensor.matmul(
                        msp[:], ssp_sb[:, lo:hi], wat_sb[:],
                        start=True, stop=True,
                    )
                    nc.tensor.matmul(
                        mbi[:], sbi_sb[0:L, lo:hi], wbt_sb[:],
                        start=True, stop=True,
                    )
                    nc.tensor.matmul(
                        nbi[:], sbi_sb[LQ - 1 : LQ, lo:hi],
                        one1[LQ - 1 : LQ, :], start=True, stop=True,
                    )
                    rbi = spool.tile([128, 1], f32, tag="rn")
                    nc.vector.reciprocal(rbi[:], nbi[:])
                    tmp = spool.tile([128, L], f32, tag="tmp")
                    nc.vector.scalar_tensor_tensor(
                        tmp[:], msp[:], rsp_sb[:, it : it + 1], unt_sb[:, it],
                        op0=MUL, op1=ADD,
                    )
                    cur = spool.tile([128, L], f32, tag="cur")
                    nc.vector.scalar_tensor_tensor(
                        cur[:], mbi[:], rbi[:], tmp[:], op0=MUL, op1=ADD
                    )
                    ex = spool.tile([128, L], f32, tag="ex")
                    ssum = spool.tile([128, 1], f32, tag="sum")
                    rsum = spool.tile([128, 1], f32, tag="rec")
                    nc.scalar.activation(ex[:], cur[:], EXP, accum_out=ssum[:])
                    nc.vector.reciprocal(rsum[:], ssum[:])
                    if last:
                        nc.vector.tensor_scalar_mul(ex[:], ex[:], rsum[:])
                        nc.sync.dma_start(outq[lo:hi, :], ex[:])
                    else:
                        exq = spool.tile([128, L], bf16, tag="exq")
                        nc.vector.tensor_scalar_mul(exq[:], ex[:], rsum[:])
                        nc.sync.dma_start(qsl[lo:hi, :], exq[:])
                if not last:
                    if sim:
                        for c in range(NUM_CORES):
                            nc.sync.dma_start(qfull[c * R : (c + 1) * R, :], qsl)
                    else:
                        nc.gpsimd.collective_compute(
                            "AllGather",
                            mybir.AluOpType.bypass,
                            replica_groups=[list(range(NUM_CORES))],
                            ins=[qsl.opt()],
                            outs=[qfull.opt()],
                        )
                    # chunked reload: matmuls on early j-tiles start while
                    # later chunks are still loading
                    qf3 = qfull.rearrange("(t p) l -> p t l", p=128)
                    for c4 in range(4):
                        nc.sync.dma_start(
                            qT[:, c4 * 16 : (c4 + 1) * 16, 0:L],
                            qf3[:, c4 * 16 : (c4 + 1) * 16, :],
                        )

    nc.compile()
    _CACHE[key] = nc
    return nc


def _host_inputs(image, logits, unary, spatial_ker_weights, bilateral_ker_weights,
                 compatibility_matrix):
    img = np.asarray(image, np.float32)[0].reshape(3, N)
    zz, yy, xx = np.meshgrid(
        np.arange(D), np.arange(H), np.arange(W), indexing="ij"
    )
    pos = np.stack([zz, yy, xx]).reshape(3, N).astype(np.float32)

    import ml_dtypes

    bf = ml_dtypes.bfloat16

    def to_bf16(x):
        return x.astype(bf).astype(np.float32)

    # two-limb bf16 split of the bilateral features: f = hi + lo
    feats = np.concatenate([pos / ALPHA, img / BETA], 0)
    fh = to_bf16(feats)
    fl = to_bf16(feats - fh)
    ft = fh + fl                     # the features the device actually uses
    sq = (ft * ft).sum(0, dtype=np.float64).astype(np.float32)
    sh = to_bf16(-0.5 * sq)          # i-side sq limbs; truncation cancels in S/n
    sl = to_bf16(-0.5 * sq - sh)
    ones = np.ones((1, N), np.float32)
    u_bi = np.concatenate([fh, fh, fl, fl, ones, ones], 0).astype(bf)
    v_bi = np.concatenate([fh, fl, fh, fl, sh[None], sl[None]], 0).astype(bf)
    # exact fp32 j-side bias, laid out (p, jtile)
    sqj_np = np.ascontiguousarray(
        (-0.5 * sq).reshape(NB, 128).T.astype(np.float32)
    )

    # separable spatial kernel: K_sp = Gz (x) Gy (x) Gx
    def g1d(n):
        a = np.arange(n, dtype=np.float64)
        return np.exp(-0.5 * ((a[:, None] - a[None, :]) / GAMMA) ** 2)

    Gz, Gy, Gx = g1d(D), g1d(H), g1d(W)
    Kp = np.kron(Gy, Gx).astype(np.float32)          # (1024, 1024) plane kernel
    kpl_np = np.ascontiguousarray(
        Kp.reshape(PT, 128, R).transpose(1, 0, 2).reshape(128, PT * R)
    ).astype(bf)
    n_sp = np.kron(Gz.sum(1), np.kron(Gy.sum(1), Gx.sum(1)))   # (N,)
    rsp_full = (1.0 / n_sp).astype(np.float32)
    Gz32 = Gz.astype(np.float32)

    cm = np.asarray(compatibility_matrix, np.float32)
    wa_t = np.ascontiguousarray((cm @ np.asarray(spatial_ker_weights, np.float32)).T)
    wb_t = np.ascontiguousarray((cm @ np.asarray(bilateral_ker_weights, np.float32)).T)
    un_t = np.ascontiguousarray(np.asarray(unary, np.float32)[0].reshape(L, N).T)
    lg_t = np.ascontiguousarray(np.asarray(logits, np.float32)[0].reshape(L, N).T)

    maps = []
    for c in range(NUM_CORES):
        cols = slice(c * R, (c + 1) * R)
        maps.append({
            "ubi": u_bi,
            "vbi": np.ascontiguousarray(v_bi[:, cols]),
            "sqj": sqj_np,
            "kpl": kpl_np,
            "gzc": np.ascontiguousarray(np.tile(Gz32[c], (128, 1))),
            "rsp": np.ascontiguousarray(
                rsp_full[cols].reshape(RT, 128).T
            ),
            "unt": np.ascontiguousarray(un_t[cols]),
            "lgt": lg_t,
            "wat": wa_t,
            "wbt": wb_t,
        })
    return maps


def kernel(**inputs):
    from concourse.bass_utils import run_bass_kernel_spmd

    nc = _build()
    in_maps = _host_inputs(**inputs)
    res = run_bass_kernel_spmd(nc, in_maps, core_ids=list(range(NUM_CORES)))
    full = np.concatenate([res.results[c]["outq"] for c in range(NUM_CORES)], 0)
    return np.ascontiguousarray(full.T).reshape(1, L, D, H, W).astype(np.float32)


# revision 9
# speedup vs baseline: 1.8014x; 1.1494x over previous
"""CRF-RNN (crfasrnn) Bass kernel for 8 Trainium2 NeuronCores — v3.

N=8192 voxels, L=21 labels, 5 mean-field iterations. Each core owns one
z-plane (R=1024 voxels) of the output columns.

Structure (see v2 notes for the separable-K_sp derivation):
- K_sp = Gz (x) Gy (x) Gx is separable: the spatial filter is a DVE
  z-mix (8 weighted adds of q tiles) + an 8-k-tile plane matmul against
  Kp = Gy (x) Gx held in SBUF. 1/n_sp is a host-side geometry constant.
- K_bi is built once by the exact two-limb bf16 gram -> ACT exp and
  cached in SBUF for all 5 iterations.

v3 scheduling/precision changes (from the v2 trace):
- K_bi build is ACT-bound (64 exp tiles ~55us) while the gram PE work is
  only half that; fusing iteration-0's bi matmuls into the build made PE
  the pipeline constraint at low p-state (PE ramps to 2.4GHz only after
  3us of CONTINUOUS execution; every ACT/PSUM stall resets it to
  0.65-1.2GHz, blowing the phase up 2.5x). v3 runs the build pure
  (PE trivially keeps pace even degraded), then iteration 0's bi filter
  runs PE-continuous at full clock.
- ubi is prefetched whole into SBUF and the big strided constant loads
  (lgt, kpl, unt) are emitted after the gram dependencies, so the first
  gram isn't stuck behind 28us of unrelated DMA.
- A dummy 8-element AllGather warms the collective channel during the
  build (the first collective otherwise pays ~30us of setup).
- The bi filter runs in fp8e4 DoubleRow (2 j-tiles per PE instruction at
  0.5 cycles/row): K_bi is stored fp8, q is split into two fp8 limbs
  (q = qh + ql), each limb a DoubleRow accumulation into the same PSUM
  group. fp8 K with ~bf16-precision q measured 4.1e-3 rel err in the
  numpy pilot (fp8 q alone is 2.9e-2 — the limb split is what makes
  this safe). The ones column rides in qh (exact 1.0; ql's is 0) so
  n_bi still falls out of PSUM row 32.

Hard-won constraints: every PE accumulation group needs its own PSUM
tensor; ACT bias APs must be 2D; matmul operands must share a dtype;
matmul base partitions must be 0/32/64 (ones row lives at partition 32).
"""

import sys

sys.path.insert(0, "/opt/trn_rl_repo")

import numpy as np

NUM_CORES = 8
GAMMA, ALPHA, BETA = 3.0, 160.0, 3.0
NUM_ITERS = 5
L = 21
D, H, W = 8, 32, 32
N = D * H * W          # 8192
R = N // NUM_CORES     # 1024 columns per core (one z-plane)
NB = N // 128          # 64 j tiles
RT = R // 128          # 8 i tiles per core
PT = R // 128          # 8 plane k-tiles
FBI = 26               # 6 bilateral features x4 hi/lo cross limbs + 2 sq rows
LQ = 34                # q rows padded: ones row (n_bi) at partition 32, even width for dual-fp8 LW
ONE = 32               # ones-column index

USE_FP8 = False

_CACHE = {}


def _build(num_iters=NUM_ITERS, sim=False, fp8=USE_FP8):
    key = ("nc", num_iters, sim, fp8)
    if key in _CACHE:
        return _CACHE[key]

    import concourse.bacc as bacc
    import concourse.mybir as mybir
    import concourse.tile as tile

    f32 = mybir.dt.float32
    bf16 = mybir.dt.bfloat16
    KDT = mybir.dt.float8e4 if fp8 else bf16
    EXP = mybir.ActivationFunctionType.Exp
    MUL = mybir.AluOpType.mult
    ADD = mybir.AluOpType.add
    SUB = mybir.AluOpType.subtract
    DR = mybir.MatmulPerfMode.DoubleRow

    nc = bacc.Bacc(
        "TRN2", target_bir_lowering=False, debug=False,
        num_devices=1 if sim else NUM_CORES,
    )

    QW = 2 * L if fp8 else L   # gathered q row width (two fp8 limbs or one bf16)
    QDT = mybir.dt.float8e4 if fp8 else bf16

    ubi = nc.dram_tensor("ubi", [FBI, N], bf16, kind="ExternalInput").ap()
    vbi = nc.dram_tensor("vbi", [FBI, R], bf16, kind="ExternalInput").ap()
    sqj = nc.dram_tensor("sqj", [128, NB], f32, kind="ExternalInput").ap()
    kpl = nc.dram_tensor("kpl", [128, PT * R], bf16, kind="ExternalInput").ap()
    gzc = nc.dram_tensor("gzc", [128, D], f32, kind="ExternalInput").ap()
    rsp = nc.dram_tensor("rsp", [128, RT], f32, kind="ExternalInput").ap()
    unt = nc.dram_tensor("unt", [R, L], f32, kind="ExternalInput").ap()
    lgt = nc.dram_tensor("lgt", [N, L], f32, kind="ExternalInput").ap()
    wat = nc.dram_tensor("wat", [L, L], f32, kind="ExternalInput").ap()
    wbt = nc.dram_tensor("wbt", [L, L], f32, kind="ExternalInput").ap()
    outq = nc.dram_tensor("outq", [R, L], f32, kind="ExternalOutput").ap()

    qsl = nc.dram_tensor("qsl", [R, QW], QDT).ap()
    qfull = nc.dram_tensor("qfull", [N, QW], QDT, addr_space="Shared").ap()
    wsrc = nc.dram_tensor("wsrc", [8, 1], bf16).ap()
    wdst = nc.dram_tensor("wdst", [8 * NUM_CORES, 1], bf16, addr_space="Shared").ap()

    with tile.TileContext(nc) as tc:
        with (
            tc.tile_pool(name="const", bufs=1) as cpool,
            tc.tile_pool(name="ssb", bufs=2) as wpool,
            tc.tile_pool(name="small", bufs=4) as spool,
            tc.tile_pool(name="gps", bufs=2, space="PSUM") as gpool,
            tc.tile_pool(name="sps", bufs=2, space="PSUM") as s_pool,
        ):
            # gram dependencies first so the build starts immediately
            vbi_sb = cpool.tile([FBI, R], bf16)
            nc.sync.dma_start(vbi_sb[:], vbi)
            sqj_sb = cpool.tile([128, NB], f32)
            nc.sync.dma_start(sqj_sb[:], sqj)
            ubi_sb = cpool.tile([FBI, N], bf16)
            nc.sync.dma_start(ubi_sb[:], ubi)

            # warm the collective channel under the build
            if not sim:
                nc.gpsimd.collective_compute(
                    "AllGather",
                    mybir.AluOpType.bypass,
                    replica_groups=[list(range(NUM_CORES))],
                    ins=[wsrc.opt()],
                    outs=[wdst.opt()],
                )

            gzc_sb = cpool.tile([128, D], f32)
            nc.sync.dma_start(gzc_sb[:], gzc)
            rsp_sb = cpool.tile([128, RT], f32)
            nc.sync.dma_start(rsp_sb[:], rsp)
            wat_sb = cpool.tile([L, L], f32)
            nc.sync.dma_start(wat_sb[:], wat)
            wbt_sb = cpool.tile([L, L], f32)
            nc.sync.dma_start(wbt_sb[:], wbt)
            kpl_sb = cpool.tile([128, PT, R], bf16)
            nc.sync.dma_start(kpl_sb[:], kpl.rearrange("p (t i) -> p t i", t=PT))
            unt_sb = cpool.tile([128, RT, L], f32)
            nc.sync.dma_start(unt_sb[:], unt.rearrange("(t p) l -> p t l", p=128))
            lg_sb = cpool.tile([128, NB, L], f32)
            nc.sync.dma_start(lg_sb[:], lgt.rearrange("(t p) l -> p t l", p=128))
            one1 = cpool.tile([LQ, 1], f32)
            nc.vector.memset(one1[:], 1.0)

            qTh = cpool.tile([128, NB, LQ], QDT)
            nc.vector.memset(qTh[:, :, L:LQ], 0.0)
            nc.vector.memset(qTh[:, :, ONE : ONE + 1], 1.0)
            if fp8:
                qTl = cpool.tile([128, NB, LQ], QDT)
                nc.vector.memset(qTl[:, :, L:LQ], 0.0)
            qsumT = cpool.tile([128, NB, L], bf16)
            kbi_st = cpool.tile([128, NB, R], KDT)

            # ---- K_bi build: ACT-bound pipeline, PE gram keeps pace ----
            for jt in range(NB):
                g = gpool.tile([128, R], f32, tag="g")
                for h in range(2):
                    nc.tensor.matmul(
                        g[:, h * 512 : (h + 1) * 512],
                        ubi_sb[:, jt * 128 : (jt + 1) * 128],
                        vbi_sb[:, h * 512 : (h + 1) * 512],
                        start=True, stop=True,
                    )
                nc.scalar.activation(
                    kbi_st[:, jt, :], g[:], EXP,
                    bias=sqj_sb[:, jt : jt + 1],
                )

            def emit_softmax0(jt):
                # iteration-0 softmax of the full logits: local, no gather.
                # DVE/Pool alternate on the elementwise tail so neither
                # becomes the phase bottleneck.
                eng = nc.vector  # Pool rejects TensorScalar at codegen
                ssum = spool.tile([128, 1], f32, tag="sum")
                rsum = spool.tile([128, 1], f32, tag="rec")
                ex = spool.tile([128, L], f32, tag="ex0")
                nc.scalar.activation(ex[:], lg_sb[:, jt], EXP, accum_out=ssum[:])
                nc.vector.reciprocal(rsum[:], ssum[:])
                eng.tensor_scalar_mul(qsumT[:, jt, :], ex[:], rsum[:])
                eng.tensor_scalar_mul(qTh[:, jt, 0:L], ex[:], rsum[:])
                if fp8:
                    eng.scalar_tensor_tensor(
                        qTl[:, jt, 0:L], ex[:], rsum[:], qTh[:, jt, 0:L],
                        op0=MUL, op1=SUB,
                    )

            def emit_bi(sbi_ps, first, jts):
                # fp8: DoubleRow over j-tile pairs, hi and lo limb groups
                # accumulate into one PSUM bracket. bf16: plain per-tile.
                if fp8:
                    pairs = [jts[i : i + 2] for i in range(0, len(jts), 2)]
                    for pr in pairs:
                        t0 = pr[0]
                        lst = pr == pairs[-1] and jts[-1] == NB - 1
                        for h in range(2):
                            for lim, qt in ((0, qTh), (1, qTl)):
                                nc.tensor.matmul(
                                    sbi_ps[:, h * 512 : (h + 1) * 512],
                                    qt[:, t0 : t0 + 2, :],
                                    kbi_st[:, t0 : t0 + 2, h * 512 : (h + 1) * 512],
                                    start=(first and t0 == 0 and h == 0 and lim == 0),
                                    stop=(lst and h == 1 and lim == 1),
                                    perf_mode=DR,
                                )
                else:
                    # start/stop are per PSUM region: each 512-column half
                    # needs its own bracket
                    for jt in jts:
                        for h in range(2):
                            nc.tensor.matmul(
                                sbi_ps[:, h * 512 : (h + 1) * 512],
                                qTh[:, jt, :],
                                kbi_st[:, jt, h * 512 : (h + 1) * 512],
                                start=(jt == 0),
                                stop=(jt == NB - 1),
                            )

            for step in range(num_iters):
                sbi_ps = s_pool.tile([LQ, R], f32, tag="s")
                if step == 0:
                    # iteration 0: softmax0 feeds the bi filter jt-pair-wise
                    for t in range(NB // 2):
                        emit_softmax0(2 * t)
                        emit_softmax0(2 * t + 1)
                        emit_bi(sbi_ps, t == 0, [2 * t, 2 * t + 1])
                else:
                    emit_bi(sbi_ps, True, list(range(NB)))

                # spatial path: z-mix on DVE (runs under the bi matmuls),
                # then the separable plane matmul (8 k-tiles)
                acc = spool.tile([128, PT, L], f32, tag="zm")
                nc.vector.tensor_scalar_mul(
                    acc[:], qsumT[:, 0:PT, :], gzc_sb[:, 0:1]
                )
                for z in range(1, D - 1):
                    nc.vector.scalar_tensor_tensor(
                        acc[:], qsumT[:, z * PT : (z + 1) * PT, :],
                        gzc_sb[:, z : z + 1], acc[:], op0=MUL, op1=ADD,
                    )
                bmix = spool.tile([128, PT, L], bf16, tag="bm")
                nc.vector.scalar_tensor_tensor(
                    bmix[:], qsumT[:, (D - 1) * PT : D * PT, :],
                    gzc_sb[:, D - 1 : D], acc[:], op0=MUL, op1=ADD,
                )
                ssp_ps = s_pool.tile([L, R], f32, tag="s")
                for kt in range(PT):
                    for h in range(2):
                        nc.tensor.matmul(
                            ssp_ps[:, h * 512 : (h + 1) * 512],
                            bmix[:, kt, :],
                            kpl_sb[:, kt, h * 512 : (h + 1) * 512],
                            start=(kt == 0), stop=(kt == PT - 1),
                        )

                ssp_sb = wpool.tile([L, R], f32, tag="ssb")
                sbi_sb = wpool.tile([LQ, R], f32, tag="ssb")
                # GPSIMD cannot access PSUM; ACT does this copy (DVE has
                # the sbi copy, so the two run in parallel)
                nc.scalar.activation(
                    ssp_sb[:], ssp_ps[:], mybir.ActivationFunctionType.Copy
                )
                nc.vector.tensor_copy(sbi_sb[:], sbi_ps[:])
                last = step == num_iters - 1
                for it in range(RT):
                    msp = s_pool.tile([128, L], f32, tag="s", name="msp")
                    mbi = s_pool.tile([128, L], f32, tag="s", name="mbi")
                    nbi = gpool.tile([128, 1], f32, tag="g")
                    lo, hi = it * 128, (it + 1) * 128
                    # fused transpose + label matmul; n_bi^T via ones column
                    nc.tensor.matmul(
                        msp[:], ssp_sb[:, lo:hi], wat_sb[:],
                        start=True, stop=True,
                    )
                    nc.tensor.matmul(
                        mbi[:], sbi_sb[0:L, lo:hi], wbt_sb[:],
                        start=True, stop=True,
                    )
                    nc.tensor.matmul(
                        nbi[:], sbi_sb[ONE : ONE + 1, lo:hi],
                        one1[ONE : ONE + 1, :], start=True, stop=True,
                    )
                    rbi = spool.tile([128, 1], f32, tag="rn")
                    nc.vector.reciprocal(rbi[:], nbi[:])
                    tmp = spool.tile([128, L], f32, tag="tmp")
                    nc.vector.scalar_tensor_tensor(
                        tmp[:], msp[:], rsp_sb[:, it : it + 1], unt_sb[:, it],
                        op0=MUL, op1=ADD,
                    )
                    cur = spool.tile([128, L], f32, tag="cur")
                    nc.vector.scalar_tensor_tensor(
                        cur[:], mbi[:], rbi[:], tmp[:], op0=MUL, op1=ADD
                    )
                    ex = spool.tile([128, L], f32, tag="ex")
                    ssum = spool.tile([128, 1], f32, tag="sum")
                    rsum = spool.tile([128, 1], f32, tag="rec")
                    nc.scalar.activation(ex[:], cur[:], EXP, accum_out=ssum[:])
                    nc.vector.reciprocal(rsum[:], ssum[:])
                    if last:
                        nc.vector.tensor_scalar_mul(ex[:], ex[:], rsum[:])
                        nc.sync.dma_start(outq[lo:hi, :], ex[:])
                    else:
                        exq = spool.tile([128, QW], QDT, tag="exq")
                        nc.vector.tensor_scalar_mul(exq[:, 0:L], ex[:], rsum[:])
                        if fp8:
                            nc.vector.scalar_tensor_tensor(
                                exq[:, L : 2 * L], ex[:], rsum[:], exq[:, 0:L],
                                op0=MUL, op1=SUB,
                            )
                        nc.sync.dma_start(qsl[lo:hi, :], exq[:])
                if not last:
                    if sim:
                        for c in range(NUM_CORES):
                            nc.sync.dma_start(qfull[c * R : (c + 1) * R, :], qsl)
                    else:
                        nc.gpsimd.collective_compute(
                            "AllGather",
                            mybir.AluOpType.bypass,
                            replica_groups=[list(range(NUM_CORES))],
                            ins=[qsl.opt()],
                            outs=[qfull.opt()],
                        )
                    # chunked reload: matmuls on early j-tiles start while
                    # later chunks are still loading
                    qf3 = qfull.rearrange("(t p) l -> p t l", p=128)
                    for c4 in range(4):
                        ch = slice(c4 * 16, (c4 + 1) * 16)
                        nc.sync.dma_start(qTh[:, ch, 0:L], qf3[:, ch, 0:L])
                        if fp8:
                            nc.sync.dma_start(
                                qTl[:, ch, 0:L], qf3[:, ch, L : 2 * L]
                            )
                    if fp8:
                        nc.vector.tensor_tensor(
                            qsumT[:], qTh[:, :, 0:L], qTl[:, :, 0:L], ADD
                        )
                    else:
                        nc.vector.tensor_copy(qsumT[:], qTh[:, :, 0:L])

    nc.compile()
    _CACHE[key] = nc
    return nc


def _host_inputs(image, logits, unary, spatial_ker_weights, bilateral_ker_weights,
                 compatibility_matrix):
    img = np.asarray(image, np.float32)[0].reshape(3, N)
    zz, yy, xx = np.meshgrid(
        np.arange(D), np.arange(H), np.arange(W), indexing="ij"
    )
    pos = np.stack([zz, yy, xx]).reshape(3, N).astype(np.float32)

    import ml_dtypes

    bf = ml_dtypes.bfloat16

    def to_bf16(x):
        return x.astype(bf).astype(np.float32)

    # two-limb bf16 split of the bilateral features: f = hi + lo
    feats = np.concatenate([pos / ALPHA, img / BETA], 0)
    fh = to_bf16(feats)
    fl = to_bf16(feats - fh)
    ft = fh + fl                     # the features the device actually uses
    sq = (ft * ft).sum(0, dtype=np.float64).astype(np.float32)
    sh = to_bf16(-0.5 * sq)          # i-side sq limbs; truncation cancels in S/n
    sl = to_bf16(-0.5 * sq - sh)
    ones = np.ones((1, N), np.float32)
    u_bi = np.concatenate([fh, fh, fl, fl, ones, ones], 0).astype(bf)
    v_bi = np.concatenate([fh, fl, fh, fl, sh[None], sl[None]], 0).astype(bf)
    # exact fp32 j-side bias, laid out (p, jtile)
    sqj_np = np.ascontiguousarray(
        (-0.5 * sq).reshape(NB, 128).T.astype(np.float32)
    )

    # separable spatial kernel: K_sp = Gz (x) Gy (x) Gx
    def g1d(n):
        a = np.arange(n, dtype=np.float64)
        return np.exp(-0.5 * ((a[:, None] - a[None, :]) / GAMMA) ** 2)

    Gz, Gy, Gx = g1d(D), g1d(H), g1d(W)
    Kp = np.kron(Gy, Gx).astype(np.float32)          # (1024, 1024) plane kernel
    kpl_np = np.ascontiguousarray(
        Kp.reshape(PT, 128, R).transpose(1, 0, 2).reshape(128, PT * R)
    ).astype(bf)
    n_sp = np.kron(Gz.sum(1), np.kron(Gy.sum(1), Gx.sum(1)))   # (N,)
    rsp_full = (1.0 / n_sp).astype(np.float32)
    Gz32 = Gz.astype(np.float32)

    cm = np.asarray(compatibility_matrix, np.float32)
    wa_t = np.ascontiguousarray((cm @ np.asarray(spatial_ker_weights, np.float32)).T)
    wb_t = np.ascontiguousarray((cm @ np.asarray(bilateral_ker_weights, np.float32)).T)
    un_t = np.ascontiguousarray(np.asarray(unary, np.float32)[0].reshape(L, N).T)
    lg_t = np.ascontiguousarray(np.asarray(logits, np.float32)[0].reshape(L, N).T)

    maps = []
    for c in range(NUM_CORES):
        cols = slice(c * R, (c + 1) * R)
        maps.append({
            "ubi": u_bi,
            "vbi": np.ascontiguousarray(v_bi[:, cols]),
            "sqj": sqj_np,
            "kpl": kpl_np,
            "gzc": np.ascontiguousarray(np.tile(Gz32[c], (128, 1))),
            "rsp": np.ascontiguousarray(
                rsp_full[cols].reshape(RT, 128).T
            ),
            "unt": np.ascontiguousarray(un_t[cols]),
            "lgt": lg_t,
            "wat": wa_t,
            "wbt": wb_t,
        })
    return maps


def kernel(**inputs):
    from concourse.bass_utils import run_bass_kernel_spmd

    nc = _build()
    in_maps = _host_inputs(**inputs)
    res = run_bass_kernel_spmd(nc, in_maps, core_ids=list(range(NUM_CORES)))
    full = np.concatenate([res.results[c]["outq"] for c in range(NUM_CORES)], 0)
    return np.ascontiguousarray(full.T).reshape(1, L, D, H, W).astype(np.float32)


# revision 10
# speedup vs baseline: 1.9162x; 1.0637x over previous
"""CRF-RNN (crfasrnn) Bass kernel for 8 Trainium2 NeuronCores — v3.

N=8192 voxels, L=21 labels, 5 mean-field iterations. Each core owns one
z-plane (R=1024 voxels) of the output columns.

Structure (see v2 notes for the separable-K_sp derivation):
- K_sp = Gz (x) Gy (x) Gx is separable: the spatial filter is a DVE
  z-mix (8 weighted adds of q tiles) + an 8-k-tile plane matmul against
  Kp = Gy (x) Gx held in SBUF. 1/n_sp is a host-side geometry constant.
- K_bi is built once by the exact two-limb bf16 gram -> ACT exp and
  cached in SBUF for all 5 iterations.

v3 scheduling/precision changes (from the v2 trace):
- K_bi build is ACT-bound (64 exp tiles ~55us) while the gram PE work is
  only half that; fusing iteration-0's bi matmuls into the build made PE
  the pipeline constraint at low p-state (PE ramps to 2.4GHz only after
  3us of CONTINUOUS execution; every ACT/PSUM stall resets it to
  0.65-1.2GHz, blowing the phase up 2.5x). v3 runs the build pure
  (PE trivially keeps pace even degraded), then iteration 0's bi filter
  runs PE-continuous at full clock.
- ubi is prefetched whole into SBUF and the big strided constant loads
  (lgt, kpl, unt) are emitted after the gram dependencies, so the first
  gram isn't stuck behind 28us of unrelated DMA.
- A dummy 8-element AllGather warms the collective channel during the
  build (the first collective otherwise pays ~30us of setup).
- The bi filter runs in fp8e4 DoubleRow (2 j-tiles per PE instruction at
  0.5 cycles/row): K_bi is stored fp8, q is split into two fp8 limbs
  (q = qh + ql), each limb a DoubleRow accumulation into the same PSUM
  group. fp8 K with ~bf16-precision q measured 4.1e-3 rel err in the
  numpy pilot (fp8 q alone is 2.9e-2 — the limb split is what makes
  this safe). The ones column rides in qh (exact 1.0; ql's is 0) so
  n_bi still falls out of PSUM row 32.

Hard-won constraints: every PE accumulation group needs its own PSUM
tensor; ACT bias APs must be 2D; matmul operands must share a dtype;
matmul base partitions must be 0/32/64 (ones row lives at partition 32).
"""

import sys

sys.path.insert(0, "/opt/trn_rl_repo")

import numpy as np

NUM_CORES = 8
GAMMA, ALPHA, BETA = 3.0, 160.0, 3.0
NUM_ITERS = 5
L = 21
D, H, W = 8, 32, 32
N = D * H * W          # 8192
R = N // NUM_CORES     # 1024 columns per core (one z-plane)
NB = N // 128          # 64 j tiles
RT = R // 128          # 8 i tiles per core
PT = R // 128          # 8 plane k-tiles
FBI = 26               # 6 bilateral features x4 hi/lo cross limbs + 2 sq rows
LQ = 34                # q rows padded: ones row (n_bi) at partition 32, even width for dual-fp8 LW
ONE = 32               # ones-column index

USE_FP8 = False

_CACHE = {}


def _build(num_iters=NUM_ITERS, sim=False, fp8=USE_FP8):
    key = ("nc", num_iters, sim, fp8)
    if key in _CACHE:
        return _CACHE[key]

    import concourse.bacc as bacc
    import concourse.mybir as mybir
    import concourse.tile as tile

    f32 = mybir.dt.float32
    bf16 = mybir.dt.bfloat16
    KDT = mybir.dt.float8e4 if fp8 else bf16
    EXP = mybir.ActivationFunctionType.Exp
    MUL = mybir.AluOpType.mult
    ADD = mybir.AluOpType.add
    SUB = mybir.AluOpType.subtract
    DR = mybir.MatmulPerfMode.DoubleRow

    nc = bacc.Bacc(
        "TRN2", target_bir_lowering=False, debug=False,
        num_devices=1 if sim else NUM_CORES,
    )

    QW = 2 * L if fp8 else L   # gathered q row width (two fp8 limbs or one bf16)
    QDT = mybir.dt.float8e4 if fp8 else bf16

    ubi = nc.dram_tensor("ubi", [FBI, N], bf16, kind="ExternalInput").ap()
    vbi = nc.dram_tensor("vbi", [FBI, R], bf16, kind="ExternalInput").ap()
    sqj = nc.dram_tensor("sqj", [128, NB], f32, kind="ExternalInput").ap()
    kpl = nc.dram_tensor("kpl", [128, PT * R], bf16, kind="ExternalInput").ap()
    gzc = nc.dram_tensor("gzc", [128, D], f32, kind="ExternalInput").ap()
    rsp = nc.dram_tensor("rsp", [128, RT], f32, kind="ExternalInput").ap()
    unt = nc.dram_tensor("unt", [R, L], f32, kind="ExternalInput").ap()
    lgt = nc.dram_tensor("lgt", [N, L], f32, kind="ExternalInput").ap()
    wat = nc.dram_tensor("wat", [L, L], f32, kind="ExternalInput").ap()
    wbt = nc.dram_tensor("wbt", [L, L], f32, kind="ExternalInput").ap()
    outq = nc.dram_tensor("outq", [R, L], f32, kind="ExternalOutput").ap()

    qslA = nc.dram_tensor("qslA", [R // 2, QW], QDT).ap()
    qslB = nc.dram_tensor("qslB", [R // 2, QW], QDT).ap()
    qfullA = nc.dram_tensor("qfullA", [N // 2, QW], QDT, addr_space="Shared").ap()
    qfullB = nc.dram_tensor("qfullB", [N // 2, QW], QDT, addr_space="Shared").ap()
    wsrc = nc.dram_tensor("wsrc", [8, 1], bf16).ap()
    wdst = nc.dram_tensor("wdst", [8 * NUM_CORES, 1], bf16, addr_space="Shared").ap()

    with tile.TileContext(nc) as tc:
        with (
            tc.tile_pool(name="const", bufs=1) as cpool,
            tc.tile_pool(name="ssb", bufs=2) as wpool,
            tc.tile_pool(name="small", bufs=4) as spool,
            tc.tile_pool(name="gps", bufs=2, space="PSUM") as gpool,
            tc.tile_pool(name="sps", bufs=2, space="PSUM") as s_pool,
        ):
            # gram dependencies first so the build starts immediately
            vbi_sb = cpool.tile([FBI, R], bf16)
            nc.sync.dma_start(vbi_sb[:], vbi)
            sqj_sb = cpool.tile([128, NB], f32)
            nc.sync.dma_start(sqj_sb[:], sqj)
            ubi_sb = cpool.tile([FBI, N], bf16)
            nc.sync.dma_start(ubi_sb[:], ubi)

            # warm the collective channel under the build
            if not sim:
                nc.gpsimd.collective_compute(
                    "AllGather",
                    mybir.AluOpType.bypass,
                    replica_groups=[list(range(NUM_CORES))],
                    ins=[wsrc.opt()],
                    outs=[wdst.opt()],
                )

            gzc_sb = cpool.tile([128, D], f32)
            nc.sync.dma_start(gzc_sb[:], gzc)
            rsp_sb = cpool.tile([128, RT], f32)
            nc.sync.dma_start(rsp_sb[:], rsp)
            wat_sb = cpool.tile([L, L], f32)
            nc.sync.dma_start(wat_sb[:], wat)
            wbt_sb = cpool.tile([L, L], f32)
            nc.sync.dma_start(wbt_sb[:], wbt)
            kpl_sb = cpool.tile([128, PT, R], bf16)
            nc.sync.dma_start(kpl_sb[:], kpl.rearrange("p (t i) -> p t i", t=PT))
            unt_sb = cpool.tile([128, RT, L], f32)
            nc.sync.dma_start(unt_sb[:], unt.rearrange("(t p) l -> p t l", p=128))
            lg_sb = cpool.tile([128, NB, L], f32)
            nc.sync.dma_start(lg_sb[:], lgt.rearrange("(t p) l -> p t l", p=128))
            one1 = cpool.tile([LQ, 1], f32)
            nc.vector.memset(one1[:], 1.0)

            qTh = cpool.tile([128, NB, LQ], QDT)
            nc.vector.memset(qTh[:, :, L:LQ], 0.0)
            nc.vector.memset(qTh[:, :, ONE : ONE + 1], 1.0)
            if fp8:
                qTl = cpool.tile([128, NB, LQ], QDT)
                nc.vector.memset(qTl[:, :, L:LQ], 0.0)
            qsumT = cpool.tile([128, NB, L], bf16)
            kbi_st = cpool.tile([128, NB, R], KDT)

            # ---- K_bi build: ACT-bound pipeline, PE gram keeps pace ----
            for jt in range(NB):
                g = gpool.tile([128, R], f32, tag="g")
                for h in range(2):
                    nc.tensor.matmul(
                        g[:, h * 512 : (h + 1) * 512],
                        ubi_sb[:, jt * 128 : (jt + 1) * 128],
                        vbi_sb[:, h * 512 : (h + 1) * 512],
                        start=True, stop=True,
                    )
                nc.scalar.activation(
                    kbi_st[:, jt, :], g[:], EXP,
                    bias=sqj_sb[:, jt : jt + 1],
                )

            def emit_softmax0(jt):
                # iteration-0 softmax of the full logits: local, no gather.
                # DVE/Pool alternate on the elementwise tail so neither
                # becomes the phase bottleneck.
                eng = nc.vector  # Pool rejects TensorScalar at codegen
                ssum = spool.tile([128, 1], f32, tag="sum")
                rsum = spool.tile([128, 1], f32, tag="rec")
                ex = spool.tile([128, L], f32, tag="ex0")
                nc.scalar.activation(ex[:], lg_sb[:, jt], EXP, accum_out=ssum[:])
                nc.vector.reciprocal(rsum[:], ssum[:])
                eng.tensor_scalar_mul(qsumT[:, jt, :], ex[:], rsum[:])
                eng.tensor_scalar_mul(qTh[:, jt, 0:L], ex[:], rsum[:])
                if fp8:
                    eng.scalar_tensor_tensor(
                        qTl[:, jt, 0:L], ex[:], rsum[:], qTh[:, jt, 0:L],
                        op0=MUL, op1=SUB,
                    )

            def emit_bi(sbi_ps, first, jts):
                # fp8: DoubleRow over j-tile pairs, hi and lo limb groups
                # accumulate into one PSUM bracket. bf16: plain per-tile.
                if fp8:
                    pairs = [jts[i : i + 2] for i in range(0, len(jts), 2)]
                    for pr in pairs:
                        t0 = pr[0]
                        lst = pr == pairs[-1] and jts[-1] == NB - 1
                        for h in range(2):
                            for lim, qt in ((0, qTh), (1, qTl)):
                                nc.tensor.matmul(
                                    sbi_ps[:, h * 512 : (h + 1) * 512],
                                    qt[:, t0 : t0 + 2, :],
                                    kbi_st[:, t0 : t0 + 2, h * 512 : (h + 1) * 512],
                                    start=(first and t0 == 0 and h == 0 and lim == 0),
                                    stop=(lst and h == 1 and lim == 1),
                                    perf_mode=DR,
                                )
                else:
                    # start/stop are per PSUM region: each 512-column half
                    # needs its own bracket
                    for jt in jts:
                        for h in range(2):
                            nc.tensor.matmul(
                                sbi_ps[:, h * 512 : (h + 1) * 512],
                                qTh[:, jt, :],
                                kbi_st[:, jt, h * 512 : (h + 1) * 512],
                                start=(jt == 0),
                                stop=(jt == NB - 1),
                            )

            for step in range(num_iters):
                sbi_ps = s_pool.tile([LQ, R], f32, tag="s")
                if step == 0:
                    # iteration 0: softmax0 feeds the bi filter jt-pair-wise
                    for t in range(NB // 2):
                        emit_softmax0(2 * t)
                        emit_softmax0(2 * t + 1)
                        emit_bi(sbi_ps, t == 0, [2 * t, 2 * t + 1])
                else:
                    # half-A j-tiles (rows 0:512 of each core's slice) first:
                    # they only need gather A, so PE starts while gather B flies
                    ja = [8 * c + tt for c in range(NUM_CORES) for tt in range(4)]
                    jb = [8 * c + 4 + tt for c in range(NUM_CORES) for tt in range(4)]
                    emit_bi(sbi_ps, True, ja + jb)

                # spatial path: z-mix on DVE (runs under the bi matmuls),
                # then the separable plane matmul (8 k-tiles)
                acc = spool.tile([128, PT, L], f32, tag="zm")
                nc.vector.tensor_scalar_mul(
                    acc[:], qsumT[:, 0:PT, :], gzc_sb[:, 0:1]
                )
                for z in range(1, D - 1):
                    nc.vector.scalar_tensor_tensor(
                        acc[:], qsumT[:, z * PT : (z + 1) * PT, :],
                        gzc_sb[:, z : z + 1], acc[:], op0=MUL, op1=ADD,
                    )
                bmix = spool.tile([128, PT, L], bf16, tag="bm")
                nc.vector.scalar_tensor_tensor(
                    bmix[:], qsumT[:, (D - 1) * PT : D * PT, :],
                    gzc_sb[:, D - 1 : D], acc[:], op0=MUL, op1=ADD,
                )
                ssp_ps = s_pool.tile([L, R], f32, tag="s")
                for kt in range(PT):
                    for h in range(2):
                        nc.tensor.matmul(
                            ssp_ps[:, h * 512 : (h + 1) * 512],
                            bmix[:, kt, :],
                            kpl_sb[:, kt, h * 512 : (h + 1) * 512],
                            start=(kt == 0), stop=(kt == PT - 1),
                        )

                ssp_sb = wpool.tile([L, R], f32, tag="ssb")
                sbi_sb = wpool.tile([LQ, R], f32, tag="ssb")
                # GPSIMD cannot access PSUM; ACT does this copy (DVE has
                # the sbi copy, so the two run in parallel)
                nc.scalar.activation(
                    ssp_sb[:], ssp_ps[:], mybir.ActivationFunctionType.Copy
                )
                nc.vector.tensor_copy(sbi_sb[:], sbi_ps[:])
                last = step == num_iters - 1
                for it in range(RT):
                    msp = s_pool.tile([128, L], f32, tag="s", name="msp")
                    mbi = s_pool.tile([128, L], f32, tag="s", name="mbi")
                    nbi = gpool.tile([128, 1], f32, tag="g")
                    lo, hi = it * 128, (it + 1) * 128
                    # fused transpose + label matmul; n_bi^T via ones column
                    nc.tensor.matmul(
                        msp[:], ssp_sb[:, lo:hi], wat_sb[:],
                        start=True, stop=True,
                    )
                    nc.tensor.matmul(
                        mbi[:], sbi_sb[0:L, lo:hi], wbt_sb[:],
                        start=True, stop=True,
                    )
                    nc.tensor.matmul(
                        nbi[:], sbi_sb[ONE : ONE + 1, lo:hi],
                        one1[ONE : ONE + 1, :], start=True, stop=True,
                    )
                    rbi = spool.tile([128, 1], f32, tag="rn")
                    nc.vector.reciprocal(rbi[:], nbi[:])
                    tmp = spool.tile([128, L], f32, tag="tmp")
                    nc.vector.scalar_tensor_tensor(
                        tmp[:], msp[:], rsp_sb[:, it : it + 1], unt_sb[:, it],
                        op0=MUL, op1=ADD,
                    )
                    cur = spool.tile([128, L], f32, tag="cur")
                    nc.vector.scalar_tensor_tensor(
                        cur[:], mbi[:], rbi[:], tmp[:], op0=MUL, op1=ADD
                    )
                    ex = spool.tile([128, L], f32, tag="ex")
                    ssum = spool.tile([128, 1], f32, tag="sum")
                    rsum = spool.tile([128, 1], f32, tag="rec")
                    nc.scalar.activation(ex[:], cur[:], EXP, accum_out=ssum[:])
                    nc.vector.reciprocal(rsum[:], ssum[:])
                    if last:
                        nc.vector.tensor_scalar_mul(ex[:], ex[:], rsum[:])
                        nc.sync.dma_start(outq[lo:hi, :], ex[:])
                    else:
                        exq = spool.tile([128, QW], QDT, tag="exq")
                        nc.vector.tensor_scalar_mul(exq[:, 0:L], ex[:], rsum[:])
                        if fp8:
                            nc.vector.scalar_tensor_tensor(
                                exq[:, L : 2 * L], ex[:], rsum[:], exq[:, 0:L],
                                op0=MUL, op1=SUB,
                            )
                        if it < 4:
                            nc.sync.dma_start(
                                qslA[it * 128 : (it + 1) * 128, :], exq[:]
                            )
                        else:
                            nc.sync.dma_start(
                                qslB[(it - 4) * 128 : (it - 3) * 128, :], exq[:]
                            )
                        if it == 3 and not sim:
                            nc.gpsimd.collective_compute(
                                "AllGather",
                                mybir.AluOpType.bypass,
                                replica_groups=[list(range(NUM_CORES))],
                                ins=[qslA.opt()],
                                outs=[qfullA.opt()],
                            )
                if not last:
                    if sim:
                        hr = R // 2
                        for c in range(NUM_CORES):
                            nc.sync.dma_start(qfullA[c * hr : (c + 1) * hr, :], qslA)
                            nc.sync.dma_start(qfullB[c * hr : (c + 1) * hr, :], qslB)
                    else:
                        nc.gpsimd.collective_compute(
                            "AllGather",
                            mybir.AluOpType.bypass,
                            replica_groups=[list(range(NUM_CORES))],
                            ins=[qslB.opt()],
                            outs=[qfullB.opt()],
                        )
                    # reload: gather A's tiles (global jt = 8c+0..3) land
                    # first so the next iteration's bi matmuls on them start
                    # while gather B is still in flight
                    qfA = qfullA.rearrange("(t p) l -> p t l", p=128)
                    qfB = qfullB.rearrange("(t p) l -> p t l", p=128)
                    for c in range(NUM_CORES):
                        nc.sync.dma_start(
                            qTh[:, 8 * c : 8 * c + 4, 0:L],
                            qfA[:, 4 * c : 4 * c + 4, :],
                        )
                    for c in range(NUM_CORES):
                        nc.sync.dma_start(
                            qTh[:, 8 * c + 4 : 8 * c + 8, 0:L],
                            qfB[:, 4 * c : 4 * c + 4, :],
                        )
                    nc.vector.tensor_copy(qsumT[:], qTh[:, :, 0:L])

    nc.compile()
    _CACHE[key] = nc
    return nc


def _host_inputs(image, logits, unary, spatial_ker_weights, bilateral_ker_weights,
                 compatibility_matrix):
    img = np.asarray(image, np.float32)[0].reshape(3, N)
    zz, yy, xx = np.meshgrid(
        np.arange(D), np.arange(H), np.arange(W), indexing="ij"
    )
    pos = np.stack([zz, yy, xx]).reshape(3, N).astype(np.float32)

    import ml_dtypes

    bf = ml_dtypes.bfloat16

    def to_bf16(x):
        return x.astype(bf).astype(np.float32)

    # two-limb bf16 split of the bilateral features: f = hi + lo
    feats = np.concatenate([pos / ALPHA, img / BETA], 0)
    fh = to_bf16(feats)
    fl = to_bf16(feats - fh)
    ft = fh + fl                     # the features the device actually uses
    sq = (ft * ft).sum(0, dtype=np.float64).astype(np.float32)
    sh = to_bf16(-0.5 * sq)          # i-side sq limbs; truncation cancels in S/n
    sl = to_bf16(-0.5 * sq - sh)
    ones = np.ones((1, N), np.float32)
    u_bi = np.concatenate([fh, fh, fl, fl, ones, ones], 0).astype(bf)
    v_bi = np.concatenate([fh, fl, fh, fl, sh[None], sl[None]], 0).astype(bf)
    # exact fp32 j-side bias, laid out (p, jtile)
    sqj_np = np.ascontiguousarray(
        (-0.5 * sq).reshape(NB, 128).T.astype(np.float32)
    )

    # separable spatial kernel: K_sp = Gz (x) Gy (x) Gx
    def g1d(n):
        a = np.arange(n, dtype=np.float64)
        return np.exp(-0.5 * ((a[:, None] - a[None, :]) / GAMMA) ** 2)

    Gz, Gy, Gx = g1d(D), g1d(H), g1d(W)
    Kp = np.kron(Gy, Gx).astype(np.float32)          # (1024, 1024) plane kernel
    kpl_np = np.ascontiguousarray(
        Kp.reshape(PT, 128, R).transpose(1, 0, 2).reshape(128, PT * R)
    ).astype(bf)
    n_sp = np.kron(Gz.sum(1), np.kron(Gy.sum(1), Gx.sum(1)))   # (N,)
    rsp_full = (1.0 / n_sp).astype(np.float32)
    Gz32 = Gz.astype(np.float32)

    cm = np.asarray(compatibility_matrix, np.float32)
    wa_t = np.ascontiguousarray((cm @ np.asarray(spatial_ker_weights, np.float32)).T)
    wb_t = np.ascontiguousarray((cm @ np.asarray(bilateral_ker_weights, np.float32)).T)
    un_t = np.ascontiguousarray(np.asarray(unary, np.float32)[0].reshape(L, N).T)
    lg_t = np.ascontiguousarray(np.asarray(logits, np.float32)[0].reshape(L, N).T)

    maps = []
    for c in range(NUM_CORES):
        cols = slice(c * R, (c + 1) * R)
        maps.append({
            "ubi": u_bi,
            "vbi": np.ascontiguousarray(v_bi[:, cols]),
            "sqj": sqj_np,
            "kpl": kpl_np,
            "gzc": np.ascontiguousarray(np.tile(Gz32[c], (128, 1))),
            "rsp": np.ascontiguousarray(
                rsp_full[cols].reshape(RT, 128).T
            ),
            "unt": np.ascontiguousarray(un_t[cols]),
            "lgt": lg_t,
            "wat": wa_t,
            "wbt": wb_t,
        })
    return maps


def kernel(**inputs):
    from concourse.bass_utils import run_bass_kernel_spmd

    nc = _build()
    in_maps = _host_inputs(**inputs)
    res = run_bass_kernel_spmd(nc, in_maps, core_ids=list(range(NUM_CORES)))
    full = np.concatenate([res.results[c]["outq"] for c in range(NUM_CORES)], 0)
    return np.ascontiguousarray(full.T).reshape(1, L, D, H, W).astype(np.float32)
